# revision 16
# baseline (speedup 1.0000x reference)
"""Trainium2 Bass kernel for the tree-structured dependency encoder.

Reference semantics (per node i, children-first topological order):
    leaf:     z_i = x_i
    internal: mult = max_c params[dep_c] * relu(z_{child_c})[None, :]   # [D, D]
              z_i  = x_i @ mult                                          # [D]
Output: z_root (root = node N-1), shape [1, D].

Strategy
--------
Only the root's ancestor cone matters (z flows child -> parent only), so the
host first prunes the graph to nodes reachable from the root (~35 of 256 for
the reference tree) and dedupes (child, dep) edges.

Column sharding across the 8 cores: cand[:, j] = params[d][:, j] * relu(z_c[j])
depends only on column j of the child z, and z_i[j] = x_i @ mult[:, j] needs
the full x_i (an input) plus column j of mult.  So core k owns columns
[128k, 128k+128) of every z with ZERO cross-core communication; the host
concatenates the 8 root shards at the end.

Per-core layout ("colT"): each needed dep matrix is stored [128 part = j,
1024 free = i'] so the per-child scale relu(z_c[j]) is a per-partition scalar.
  edge 1:    acc = param * s          (ACT: activation Copy with scale AP)
  edge 2..k: acc = (param * s) max acc  (DVE: fused scalar_tensor_tensor)
  GEMV:      xb  = ones^T @ x_i       (PE:  K=1 matmul broadcast into PSUM)
             z_i = sum_f(acc * xb)    (DVE: fused tensor_tensor_reduce)
  relu:      s_i = relu(z_i)          (ACT, [128,1])
Params for the needed labels are DMA'd once into SBUF (first-use order) and
reused by every edge; leaves' relu(x) scales are precomputed on the host.
"""

import numpy as np

N_CORES = 8
D = 1024
DC = D // N_CORES  # 128 columns per core

# "f32": exact (rel err ~1e-6).  "bf16": params/acc/x in bf16 with f32
# accumulation (rel err ~3e-3) — DVE hits 2x mode and DMA bytes halve.
PRECISION = "bf16"

_CACHE = {}


def _schedule(children_idx, children_dep, children_mask):
    """Prune to the root's ancestor cone and build the edge schedule."""
    n = children_idx.shape[0]
    root = n - 1
    ci = np.asarray(children_idx, dtype=np.int64)
    cd = np.asarray(children_dep, dtype=np.int64)
    cm = np.asarray(children_mask, dtype=bool)

    needed = set()
    stack = [root]
    while stack:
        i = stack.pop()
        if i in needed:
            continue
        needed.add(i)
        for c in range(ci.shape[1]):
            if cm[i, c]:
                stack.append(int(ci[i, c]))

    order = sorted(needed)  # ascending index == topological (children first)
    loc = {node: idx for idx, node in enumerate(order)}
    internal, leaves = [], []
    edges = {}
    for i in order:
        if not cm[i].any():
            leaves.append(i)
            continue
        internal.append(i)
        seen = set()
        elist = []
        for c in range(ci.shape[1]):
            if cm[i, c]:
                key = (int(ci[i, c]), int(cd[i, c]))
                if key not in seen:  # duplicate (child, dep) can't change max
                    seen.add(key)
                    elist.append(key)
        edges[i] = elist

    labels = []  # global label ids, in first-use order
    lab2slot = {}
    for i in internal:
        for _, d in edges[i]:
            if d not in lab2slot:
                lab2slot[d] = len(labels)
                labels.append(d)

    return {
        "root": root,
        "order": order,
        "loc": loc,
        "internal": internal,
        "leaves": leaves,
        "edges": edges,
        "labels": labels,
        "lab2slot": lab2slot,
    }


def _legalize_single_wait(nc):
    """Split multi-wait instructions: this walrus allows 1 sync wait/inst.

    Extra waits move to single-wait InstNoOps inserted just before the
    instruction on the same engine queue (per-engine program order is
    preserved, so the AND-semantics of the wait list is unchanged).
    """
    from concourse import mybir

    for bb in nc.main_func.blocks:
        new_list = []
        for inst in bb.instructions:
            si = inst.sync_info
            if si is not None and si.on_wait and len(si.on_wait) > 1:
                waits = list(si.on_wait)
                for w in waits[:-1]:
                    nop = mybir.InstNoOp(
                        name=nc.get_next_instruction_name(), ins=[], outs=[]
                    )
                    nop.engine = inst.engine
                    nop.sync_info = mybir.SyncInfo(on_wait=[w], on_update=[])
                    new_list.append(nop)
                inst.sync_info = mybir.SyncInfo(
                    on_wait=[waits[-1]], on_update=list(si.on_update)
                )
            new_list.append(inst)
        bb.instructions = new_list


def _build_program(sched, legalize=True):
    import concourse.bass as bass
    import concourse.tile as tile
    from concourse import mybir

    f32 = mybir.dt.float32
    MUL = mybir.AluOpType.mult
    MAX = mybir.AluOpType.max
    ADD = mybir.AluOpType.add
    RELU = mybir.ActivationFunctionType.Relu

    order = sched["order"]
    loc = sched["loc"]
    internal = sched["internal"]
    leaves = sched["leaves"]
    edges = sched["edges"]
    labels = sched["labels"]
    lab2slot = sched["lab2slot"]
    root = sched["root"]

    n_needed = len(order)

    n_leaves = max(len(leaves), 1)
    n_labels = len(labels)

    nc = bass.Bass()
    pt = nc.dram_tensor("pt", [n_labels, DC, D], f32, kind="ExternalInput")
    ec = nc.dram_tensor("ec", [n_needed, D], f32, kind="ExternalInput")
    rl = nc.dram_tensor("rl", [DC, n_leaves], f32, kind="ExternalInput")
    zr = nc.dram_tensor("zr", [DC, 1], f32, kind="ExternalOutput")

    with tile.TileContext(nc) as tc:
        with (
            tc.tile_pool(name="pparams", bufs=1) as ppool,
            tc.tile_pool(name="pwork", bufs=3) as wpool,
            tc.tile_pool(name="psmall", bufs=1) as spool,
            tc.tile_pool(name="ppsum", bufs=2, space="PSUM") as psum_pool,
        ):
            ones = spool.tile([1, DC], f32, tag="ones")
            nc.vector.memset(ones, 1.0)

            rl_t = spool.tile([DC, n_leaves], f32, tag="rl")
            nc.sync.dma_start(out=rl_t, in_=rl[:, :])

            pt_t = []
            for s in range(n_labels):
                p = ppool.tile([DC, D], f32, tag=f"p{s}")
                nc.sync.dma_start(out=p, in_=pt[s])
                pt_t.append(p)

            rel = {}
            for li, leaf in enumerate(leaves):
                rel[leaf] = rl_t[:, li : li + 1]

            z_root = None
            for i in internal:
                elist = edges[i]
                c0, d0 = elist[0]
                acc = wpool.tile([DC, D], f32, tag="acc")
                nc.gpsimd.tensor_scalar_mul(acc, pt_t[lab2slot[d0]], rel[c0])
                for c, d in elist[1:]:
                    nc.vector.scalar_tensor_tensor(
                        out=acc,
                        in0=pt_t[lab2slot[d]],
                        scalar=rel[c],
                        in1=acc,
                        op0=MUL,
                        op1=MAX,
                    )
                il = loc[i]
                xst = wpool.tile([1, D], f32, tag="xst", bufs=4)
                nc.sync.dma_start(out=xst, in_=ec[il : il + 1, :])
                xb = psum_pool.tile([DC, D], f32, tag="xb")
                nc.tensor.matmul(xb[:, 0:512], ones, xst[:, 0:512])
                nc.tensor.matmul(xb[:, 512:D], ones, xst[:, 512:D])
                scr = wpool.tile([DC, D], f32, tag="scr")
                zt = spool.tile([DC, 1], f32, tag=f"z{i}")
                nc.vector.scalar_tensor_tensor(
                    out=scr,
                    in0=acc,
                    scalar=1.0,
                    in1=xb,
                    op0=MUL,
                    op1=MUL,
                    accum_out=zt,
                )
                if i == root:
                    z_root = zt
                else:
                    rt = spool.tile([DC, 1], f32, tag=f"r{i}")
                    nc.gpsimd.tensor_scalar_max(rt, zt, 0.0)
                    rel[i] = rt

            nc.sync.dma_start(out=zr[:, :], in_=z_root)

    if legalize:
        _legalize_single_wait(nc)
    return nc


def _prepare(embeddings, params, children_idx, children_dep, children_mask,
             legalize=True):
    emb = np.ascontiguousarray(np.asarray(embeddings, dtype=np.float32))
    par = np.asarray(params, dtype=np.float32)
    sched = _schedule(children_idx, children_dep, children_mask)

    key = (
        legalize,
        tuple(sched["order"]),
        tuple(sched["labels"]),
        tuple((i, tuple(e)) for i, e in sched["edges"].items()),
    )
    if key in _CACHE:
        nc = _CACHE[key]
    else:
        nc = _build_program(sched, legalize=legalize)
        _CACHE[key] = nc

    order = sched["order"]
    leaves = sched["leaves"]
    labels = sched["labels"]
    n_needed = len(order)

    n_leaves = max(len(leaves), 1)

    # ec: needed-node embedding rows (full D), same for all cores
    ec = np.ascontiguousarray(emb[order])

    # per-core param shards, colT layout: pt[l, j, i'] = params[lab, i', 128k+j]
    p_used = par[labels]  # [L, D, D]
    in_maps = []
    for k in range(N_CORES):
        cols = slice(k * DC, (k + 1) * DC)
        pt_k = np.ascontiguousarray(p_used[:, :, cols].transpose(0, 2, 1))
        rl_k = np.zeros((DC, n_leaves), dtype=np.float32)
        if leaves:
            rl_k[:, : len(leaves)] = np.maximum(emb[leaves][:, cols], 0.0).T
        in_maps.append({"pt": pt_k, "ec": ec, "rl": rl_k})
    return sched, nc, in_maps


def _run(embeddings, params, children_idx, children_dep, children_mask, trace=False):
    emb = np.asarray(embeddings, dtype=np.float32)
    cm = np.asarray(children_mask, dtype=bool)
    root = emb.shape[0] - 1
    if not cm[root].any():  # degenerate: root is a leaf
        return emb[root : root + 1].copy(), None

    from concourse.bass_utils import run_bass_kernel_spmd

    sched, nc, in_maps = _prepare(
        embeddings, params, children_idx, children_dep, children_mask
    )
    bkr = run_bass_kernel_spmd(
        nc, in_maps, core_ids=list(range(N_CORES)), trace=trace
    )
    out = np.concatenate(
        [bkr.results[k]["zr"].reshape(DC) for k in range(N_CORES)]
    ).reshape(1, D)
    return out.astype(np.float32), bkr


def kernel(embeddings, params, children_idx, children_dep, children_mask):
    out, _ = _run(embeddings, params, children_idx, children_dep, children_mask)
    return out


def run_traced(embeddings, params, children_idx, children_dep, children_mask):
    return _run(
        embeddings, params, children_idx, children_dep, children_mask, trace=True
    )
